# revision 1
# baseline (speedup 1.0000x reference)
"""Trainium2 Bass kernel for nn_MoEFFN_86895778333203.

Dense MoE FFN: B=4, S=2048, D=512, F=2048, E=8 routed experts + 1 shared
expert, gating-weighted combine.

Sharding: data-parallel over tokens. The 8192 tokens are split 1024/core
across 8 NeuronCores; every core runs all 9 "experts" (8 routed + the
shared expert folded in as expert #8) on its token slice. No collectives.

Perf notes (HW-measured): concurrent weight DMA into SBUF is the main MM
throughput limiter (bf16 N=512 matmuls stream at ~140-190 ns without DMA,
230+ with), so the first NRES experts' weights are SBUF-resident — loaded
once in the prologue, never re-streamed. Redundant LDWEIGHTS are elided at
the BIR level (walrus's ldw-opt crashes on this toolchain). The timing rep
loop uses staggered semaphore resets + branch-prefetch hints so the PE
isn't drained at every back edge.

Per-core device algorithm (all matmuls bf16 with fp32 PSUM accumulation):
  hT_e = gelu_tanh(W1_e^T x^T + b1_e)        # [F,Tc] layout, f on partitions
  y_e  = hT_e^T W2_e + b2_e                  # [Tc,D], t on partitions
  out  = sum_e w_e * y_e                     # w_e per-token combine weights
where w_e = (1-sg)*g_e for routed experts and w_8 = sg for the shared one
(so biases b1/b2 rows 0..7 are the expert biases, row 8 = bs1/bs2).
"""

import numpy as np
import ml_dtypes

import concourse.bass as bass
import concourse.tile as tile
from concourse import mybir

# ---------------------------------------------------------------------------
# Patch TileContext._drain_and_barrier: the stock version attaches one sem
# wait per live logical proc to a single Drain instruction; this walrus
# build caps sync-wait commands per instruction, so split the waits across
# several drains (each observes <=CHUNK procs; same-engine program order
# makes the union equivalent).
# ---------------------------------------------------------------------------
from concourse.vector_clock import ScopedClock, VectorClock

_DRAIN_CHUNK = 4


def _split_drain_and_barrier(self, tick_clock, wait_clock):
    gc = tick_clock.global_clock
    n = len(gc)
    for s in range(0, n, _DRAIN_CHUNK):
        vec = [0] * n
        nonzero = False
        for i in range(s, min(s + _DRAIN_CHUNK, n)):
            vec[i] = gc[i]
            nonzero = nonzero or gc[i] > 0
        if not nonzero:
            continue
        drain_inst = self.nc.sync.drain()
        wait_clock.add_sem_waits(drain_inst.ins, ScopedClock({None: VectorClock(vec)}))
    self.nc.all_engine_barrier()
    assert self.sems is not None
    popped = self.nc._tile_sem_poison_stack.pop()
    assert popped is self._sem_poison
    self.nc.clear_and_free_semaphores(list(self.sems.allocated().values()))
    self.nc.all_engine_barrier()


tile.TileContext._drain_and_barrier = _split_drain_and_barrier


def _split_excess_waits(nc):
    """This walrus build allows at most 1 sync-wait command per instruction
    (2 for EventSemaphore ops). Tile attaches up to ~4. Hoist the excess
    onto standalone EventSemaphore wait instructions inserted immediately
    before the owner in the same block (same engine => program order is
    preserved, semantics identical)."""
    uid = 0
    for fn in nc.m.functions:
        for bb in fn.blocks:
            il = bb.instructions
            i = 0
            while i < len(il):
                inst = il[i]
                si = inst.sync_info
                waits = list(si.on_wait) if si and si.on_wait else []
                cap = 2 if isinstance(inst, mybir.InstEventSemaphore) else 1
                if len(waits) > cap:
                    keep = waits[-cap:]
                    extra = waits[:-cap]
                    new_insts = []
                    for j in range(0, len(extra), 2):
                        uid += 1
                        new_insts.append(
                            mybir.InstEventSemaphore(
                                name=f"bass_splitwait_{uid}",
                                engine=inst.engine,
                                sync_info=mybir.SyncInfo(
                                    on_wait=list(extra[j : j + 2]), on_update=[]
                                ),
                            )
                        )
                    si.on_wait = keep
                    for k, wi in enumerate(new_insts):
                        il.insert(i + k, wi)
                    i += len(new_insts)
                i += 1

def _elide_redundant_ldweights(nc):
    """Drop an InstLdweights that reloads the exact stationary operand the PE
    already holds (same memref/offset/ap/dtype, only matmuls in between, no
    sync attached). walrus's own --enable-ldw-opt pass does this but crashes
    on this toolchain, so do it on the BIR directly: a matmul with no fresh
    LDWEIGHTS keeps using the current foreground weights."""
    for fn in nc.m.functions:
        for bb in fn.blocks:
            il = bb.instructions
            keep = []
            last_key = None
            for inst in il:
                nm = type(inst).__name__
                if nm == "InstLdweights":
                    ap = inst.ins[0]
                    key = (ap.memref, ap.offset, str(ap.ap), str(ap.dtype))
                    si = inst.sync_info
                    clean = not (si and (si.on_wait or si.on_update))
                    if key == last_key and clean:
                        continue  # elide
                    last_key = key
                elif nm != "InstMatmult" and str(inst.engine).endswith("PE"):
                    last_key = None
                keep.append(inst)
            if len(keep) != len(il):
                il[:] = keep


# ---------------------------------------------------------------------------
# Problem shapes (hardcoded per contract)
# ---------------------------------------------------------------------------
B, S, D, F, E = 4, 2048, 512, 2048, 8
NCORES = 8
NTOK = B * S              # 8192 tokens total
T = NTOK // NCORES        # 1024 tokens per core
NE = E + 1                # 8 routed + shared
P = 128
DT = D // P               # 4  k-tiles for GEMM1
FT = F // P               # 16 k-tiles for GEMM2 / m-tiles for GEMM1
TCHUNK = 512              # GEMM1 rhs free-dim chunk
NTC = T // TCHUNK         # 2 chunks
TSUB = TCHUNK // P        # 4 t-tiles of 128 per chunk
NTT = T // P              # 8 t-tiles per core
NRES = 2                  # experts with SBUF-resident weights (no per-rep DMA)
NW2RES = 6                # experts 0..NW2RES-1 additionally keep W2 resident

BF16 = mybir.dt.bfloat16
F32 = mybir.dt.float32
GELU = mybir.ActivationFunctionType.Gelu_apprx_tanh

# b2/bs2 are folded in on the host (exact fp32 post-add of wc @ b2all),
# so the device kernel skips the 72 k=1 bias matmuls.
# NOTE: --enable-ldw-opt=true makes walrus crash in visitInstLdweights on
# this toolchain (which is why concourse hardcodes it off) — keep False.
LDW_OPT = False


def _patch_ldw_opt():
    """compile_bir_kernel hardcodes --enable-ldw-opt=false; rewrite it on the
    walrus_driver command line so redundant LDWEIGHTS (consecutive matmuls
    sharing a stationary operand) are elided."""
    if not LDW_OPT or getattr(_patch_ldw_opt, "_done", False):
        return
    import subprocess

    _orig_run = subprocess.run

    def _run(cmd, *args, **kwargs):
        if isinstance(cmd, (list, tuple)) and any("walrus_driver" in str(c) for c in cmd):
            cmd = [
                "--enable-ldw-opt=true" if str(c) == "--enable-ldw-opt=false" else c
                for c in cmd
            ]
        return _orig_run(cmd, *args, **kwargs)

    subprocess.run = _run
    _patch_ldw_opt._done = True


def _build_program(reps: int = 1):
    nc = bass.Bass()
    xT = nc.declare_dram_parameter("xT", [D, T], BF16, isOutput=False)
    W1 = nc.declare_dram_parameter("W1", [NE, D, F], BF16, isOutput=False)
    W2 = nc.declare_dram_parameter("W2", [NE, F, D], BF16, isOutput=False)
    WC = nc.declare_dram_parameter("WC", [T, NE], F32, isOutput=False)
    B1 = nc.declare_dram_parameter("B1", [F, NE], F32, isOutput=False)
    OUT = nc.declare_dram_parameter("OUT", [T, D], F32, isOutput=True)

    with tile.TileContext(nc) as tc:
        with (
            tc.tile_pool(name="const", bufs=1) as const_pool,
            tc.tile_pool(name="w1p", bufs=1) as w1_pool,
            tc.tile_pool(name="w2p", bufs=1) as w2_pool,
            tc.tile_pool(name="hp", bufs=20) as h_pool,
            tc.tile_pool(name="accp", bufs=1) as acc_pool,
            tc.tile_pool(name="tmpp", bufs=1) as tmp_pool,
            tc.tile_pool(name="ph", bufs=6, space="PSUM") as ph_pool,
            tc.tile_pool(name="py", bufs=2, space="PSUM") as py_pool,
        ):
            # ---- persistent staging ----
            xT_sb = const_pool.tile([P, DT, T], BF16)
            nc.sync.dma_start(xT_sb[:], xT.rearrange("(dt p) t -> p dt t", p=P))

            wc_sb = const_pool.tile([P, NTT, NE], F32)
            nc.sync.dma_start(wc_sb[:], WC.rearrange("(tt p) e -> p tt e", p=P))

            b1_sb = const_pool.tile([P, FT, NE], F32)
            nc.sync.dma_start(b1_sb[:], B1.rearrange("(ft p) e -> p ft e", p=P))

            # Weights for the first NRES experts stay SBUF-resident (loaded
            # once, outside the rep loop): concurrent weight DMA measurably
            # stalls the PE's SBUF streaming, so every byte not re-streamed
            # per rep is MM throughput back.
            w1_res = [
                const_pool.tile([P, DT, F], BF16, name=f"w1res{i}")
                for i in range(NRES)
            ]
            w2_res = [
                const_pool.tile([P, FT, D], BF16, name=f"w2res{i}")
                for i in range(NW2RES)
            ]
            for e in range(NRES):
                nc.sync.dma_start(
                    w1_res[e][:], W1[e].rearrange("(dt p) f -> p dt f", p=P)
                )
            for e in range(NW2RES):
                nc.sync.dma_start(
                    w2_res[e][:], W2[e].rearrange("(ft p) d -> p ft d", p=P)
                )

            acc = acc_pool.tile([P, NTT, D], F32)

            # ---- expert loop ----
            def expert_loop(_iv):
                for e in range(NE):
                    if e < NRES:
                        w1sb = w1_res[e]
                    else:
                        w1sb = w1_pool.tile([P, DT, F], BF16, tag="w1sb")
                        nc.sync.dma_start(
                            w1sb[:], W1[e].rearrange("(dt p) f -> p dt f", p=P)
                        )
                    if e < NW2RES:
                        w2sb = w2_res[e]
                    else:
                        w2sb = w2_pool.tile([P, FT, D], BF16, tag="w2sb")
                        nc.sync.dma_start(
                            w2sb[:], W2[e].rearrange("(ft p) d -> p ft d", p=P)
                        )

                    # Per t-chunk: GEMM1 then GEMM2, so only one chunk's 16 h
                    # tiles are live at a time (frees 12 KB/partition of h pool
                    # for another resident W2). The extra per-chunk LDWEIGHTS
                    # this costs are fully hidden (HW-measured ~0 ns in dense
                    # MM streams).
                    for tci in range(NTC):
                        # GEMM1: hT[f, t] = gelu(W1^T xT + b1), f on partitions.
                        # ft processed in pairs with the two accumulations
                        # interleaved MM-by-MM across two PSUM banks — the
                        # fastest measured accumulation pattern (same-bank
                        # runs of 4+ issue measurably slower).
                        h_tiles = [None] * FT
                        for fp in range(FT // 2):
                            fa, fb = 2 * fp, 2 * fp + 1
                            pha = ph_pool.tile([P, TCHUNK], F32, tag="ph", name="pha")
                            phb = ph_pool.tile([P, TCHUNK], F32, tag="ph", name="phb")
                            for dt in range(DT):
                                nc.tensor.matmul(
                                    pha[:],
                                    w1sb[:, dt, bass.ts(fa, P)],
                                    xT_sb[:, dt, bass.ts(tci, TCHUNK)],
                                    start=(dt == 0),
                                    stop=(dt == DT - 1),
                                )
                                nc.tensor.matmul(
                                    phb[:],
                                    w1sb[:, dt, bass.ts(fb, P)],
                                    xT_sb[:, dt, bass.ts(tci, TCHUNK)],
                                    start=(dt == 0),
                                    stop=(dt == DT - 1),
                                )
                            for ft, ph in ((fa, pha), (fb, phb)):
                                hsb = h_pool.tile([P, TCHUNK], BF16, tag="hsb")
                                nc.scalar.activation(
                                    hsb[:], ph[:], GELU, bias=b1_sb[:, ft, e : e + 1]
                                )
                                h_tiles[ft] = hsb

                        # GEMM2 + combine: t on partitions
                        for tsi in range(TSUB):
                            tt = tci * TSUB + tsi
                            py = py_pool.tile([P, D], F32, tag="py")
                            for ft in range(FT):
                                nc.tensor.matmul(
                                    py[:],
                                    h_tiles[ft][:, bass.ts(tsi, P)],
                                    w2sb[:, ft, :],
                                    start=(ft == 0),
                                    stop=(ft == FT - 1),
                                )
                            wap = wc_sb[:, tt, e : e + 1]
                            if e == 0:
                                nc.scalar.mul(acc[:, tt, :], py[:], wap)
                            else:
                                tmp = tmp_pool.tile([P, D], F32, tag="tmp")
                                nc.scalar.mul(tmp[:], py[:], wap)
                                nc.vector.tensor_add(
                                    acc[:, tt, :], acc[:, tt, :], tmp[:]
                                )

            if reps == 1:
                expert_loop(0)
            else:
                # staggered_reset: no drain + all-engine barrier on the back
                # edge (PE keeps streaming, HAM stays warm); hint_engines:
                # branch-prefetch the back-edge target (body >> one IRAM
                # block, an unhinted back edge stalls ~3-4us on I$ fetch).
                with tc.For_i(
                    0,
                    reps,
                    1,
                    hint_engines=tuple(mybir.ALL_ENGINES),
                    staggered_reset=True,
                ) as iv:
                    expert_loop(iv)

            # ---- writeback ----
            for tt in range(NTT):
                nc.sync.dma_start(OUT[bass.ts(tt, P), :], acc[:, tt, :])

    _elide_redundant_ldweights(nc)
    _split_excess_waits(nc)
    return nc


_CACHE = {}


def _make_sharded(nc):
    """Wrap a built Bass program in a cached, sharded, jitted executor."""
    import jax
    from jax.sharding import Mesh, PartitionSpec
    from jax.experimental.shard_map import shard_map
    from concourse import bass2jax

    bass2jax.install_neuronx_cc_hook()

    partition_name = nc.partition_id_tensor.name if nc.partition_id_tensor else None
    in_names = []
    out_names = []
    out_avals = []
    zero_outs = []
    for alloc in nc.m.functions[0].allocations:
        if not isinstance(alloc, mybir.MemoryLocationSet):
            continue
        name = alloc.memorylocations[0].name
        if alloc.kind == "ExternalInput":
            if name != partition_name:
                in_names.append(name)
        elif alloc.kind == "ExternalOutput":
            out_names.append(name)
            shape = tuple(alloc.tensor_shape)
            dtype = mybir.dt.np(alloc.dtype)
            out_avals.append(jax.core.ShapedArray(shape, dtype))
            zero_outs.append(np.zeros(shape, dtype))
    n_params = len(in_names)
    n_outs = len(out_avals)
    all_names = in_names + out_names
    if partition_name is not None:
        all_names = all_names + [partition_name]

    def _body(*args):
        operands = list(args)
        if partition_name is not None:
            operands.append(bass2jax.partition_id_tensor())
        outs = bass2jax._bass_exec_p.bind(
            *operands,
            out_avals=tuple(out_avals),
            in_names=tuple(all_names),
            out_names=tuple(out_names),
            lowering_input_output_aliases=(),
            sim_require_finite=True,
            sim_require_nnan=True,
            nc=nc,
        )
        return tuple(outs)

    devices = jax.devices()[:NCORES]
    mesh = Mesh(np.asarray(devices), ("core",))
    in_specs = (PartitionSpec("core"),) * (n_params + n_outs)
    out_specs = (PartitionSpec("core"),) * n_outs
    donate = tuple(range(n_params, n_params + n_outs))
    sharded = jax.jit(
        shard_map(
            _body, mesh=mesh, in_specs=in_specs, out_specs=out_specs, check_rep=False
        ),
        donate_argnums=donate,
        keep_unused=True,
    )

    def runner(in_maps, timeit=False):
        per_core = [[np.asarray(m[nm]) for nm in in_names] for m in in_maps]
        concat_in = [
            np.concatenate([per_core[c][i] for c in range(NCORES)], axis=0)
            for i in range(n_params)
        ]
        concat_zeros = [
            np.zeros((NCORES * z.shape[0], *z.shape[1:]), z.dtype) for z in zero_outs
        ]
        out_arrs = sharded(*concat_in, *concat_zeros)
        return [
            {
                nm: np.asarray(out_arrs[i]).reshape(NCORES, *out_avals[i].shape)[c]
                for i, nm in enumerate(out_names)
            }
            for c in range(NCORES)
        ]

    return runner, (in_names, out_names, out_avals, zero_outs, sharded, mesh)


def _get_runner():
    """Compile once; return a callable(list_of_in_maps) -> list_of_out_maps."""
    if "runner" in _CACHE:
        return _CACHE["runner"]
    _patch_ldw_opt()
    nc = _build_program()
    runner, meta = _make_sharded(nc)
    _CACHE["runner"] = runner
    _CACHE["nc"] = nc
    _CACHE["meta"] = meta
    return runner


def _prep_in_maps(
    hidden_states, gating_probs, shared_gate_prob, W1, b1, W2, b2, Ws1, bs1, Ws2, bs2
):
    bf16 = ml_dtypes.bfloat16
    x = np.asarray(hidden_states, np.float32).reshape(NTOK, D)
    g = np.asarray(gating_probs, np.float32).reshape(NTOK, E)
    sg = np.asarray(shared_gate_prob, np.float32).reshape(NTOK, 1)

    # combine weights: routed experts get (1-sg)*g_e, shared expert gets sg
    wc = np.concatenate([(1.0 - sg) * g, sg], axis=1).astype(np.float32)  # [NTOK, 9]

    W1all = np.concatenate(
        [np.asarray(W1, np.float32), np.asarray(Ws1, np.float32)[None]], axis=0
    ).astype(bf16)  # [9, D, F]
    W2all = np.concatenate(
        [np.asarray(W2, np.float32), np.asarray(Ws2, np.float32)[None]], axis=0
    ).astype(bf16)  # [9, F, D]
    B1all = (
        np.concatenate(
            [np.asarray(b1, np.float32), np.asarray(bs1, np.float32)[None]], axis=0
        )
        .T.astype(np.float32)
        .copy()
    )  # [F, 9]
    B2all = np.concatenate(
        [np.asarray(b2, np.float32), np.asarray(bs2, np.float32)[None]], axis=0
    )  # [9, D] — folded on the host: OUT += wc @ B2all (exact fp32)

    in_maps = []
    for c in range(NCORES):
        sl = slice(c * T, (c + 1) * T)
        in_maps.append(
            {
                "xT": np.ascontiguousarray(x[sl].T).astype(bf16),
                "W1": W1all,
                "W2": W2all,
                "WC": np.ascontiguousarray(wc[sl]),
                "B1": B1all,
            }
        )
    return in_maps, wc @ B2all


def kernel(**inputs) -> np.ndarray:
    runner = _get_runner()
    in_maps, bias_img = _prep_in_maps(**inputs)
    results = runner(in_maps)
    out = np.concatenate(
        [np.asarray(results[c]["OUT"], np.float32) for c in range(NCORES)], axis=0
    )
    out += bias_img
    return out.reshape(B, S, D)

